# revision 27
# baseline (speedup 1.0000x reference)
"""2-layer GAT on 8 Trainium2 NeuronCores — descriptor-lean edition.

Gathers are SWDGE-descriptor-latency-bound (~4.4ns/row regardless of row
size), so the design minimizes gathered rows and hides everything else
under the gather stream:

- dst nodes dealt globally by degree: lo-original nodes (id<25000) across
  cores 0-3, hi across 4-7, so all 8 cores share one tile geometry with
  minimal max-degree padding.
- self-loops are never gathered: their contribution is computed analytically
  from the local pre-AllGather shard (sh1/sh2) and injected as an extra
  slot in the per-tile weighted-sum reduce.
- softmax denominator fused into the payload reduce via a ones-channel in
  the table row: L1 row = 4x[h(32)|1] + as(4) -> 512B; L2 row =
  [h2(64)|1|as2] -> 256B.
- per-tile compute uses contiguous-inner APs: multiply writes [p,k,(h,33)]
  channel-inner; the reduce runs strided over k.
- pad-node safety: pad rows carry as=-1e30 so their softmax weights vanish;
  self-weights are clamped to >=1e-16 so pad denominators stay finite.
"""

import numpy as np

N = 50000
E = 800000
R = 8
NPC = N // R  # 6250 owned nodes per core
TPC = 49  # tiles of 128 nodes
NPAD = TPC * 128  # 6272 rows per shard
HALF = 4 * NPAD  # 25088 table rows per half
IN_CH = 128
HIDDEN = 32
HEADS = 4
OUT_CH = 64
NEG_SLOPE = 0.2
EL1 = 256  # L1 table row, bf16 elems: [4x(32h|1)=132 | as(4) | pad], 512B
EL2 = 128  # L2 table row, bf16 elems: [h2(64) | 1 | as2 | pad], 256B
SENT = 6250  # sentinel row (first pad row of core 0 / core 4) in each half
RW1 = 136  # sh1 row width: 132 + as(4)
RW2 = 66  # sh2 row width: 64 + 1 + as2


# ---------------------------------------------------------------- host planner
def _build_plan(edge_index):
    src = np.asarray(edge_index[0], dtype=np.int64)
    dst = np.asarray(edge_index[1], dtype=np.int64)
    half_of = (src >= N // 2).astype(np.int64)  # table half by ORIGINAL id

    d_lo = np.bincount(dst[half_of == 0], minlength=N)
    d_hi = np.bincount(dst[half_of == 1], minlength=N)

    # global degree sort + deal: lo-original ids across cores 0-3, hi 4-7
    key = (d_lo // 3) * 10000 + d_hi
    perms = [None] * R
    pos = np.empty(N, dtype=np.int64)
    for base, ids in ((0, np.arange(N // 2)), (4, np.arange(N // 2, N))):
        order = ids[np.argsort(-key[ids], kind="stable")]
        for c4 in range(4):
            p = order[c4::4]
            perms[base + c4] = p
            pos[p] = (base + c4) * NPAD + np.arange(NPC)

    # shared per-tile max degrees (over all 8 cores; pad rows deg 0)
    dlo_t = np.zeros(TPC, dtype=np.int64)
    dhi_t = np.zeros(TPC, dtype=np.int64)
    for c in range(R):
        dl = np.concatenate([d_lo[perms[c]], np.zeros(NPAD - NPC, dtype=np.int64)])
        dh = np.concatenate([d_hi[perms[c]], np.zeros(NPAD - NPC, dtype=np.int64)])
        dlo_t = np.maximum(dlo_t, dl.reshape(TPC, 128).max(axis=1))
        dhi_t = np.maximum(dhi_t, dh.reshape(TPC, 128).max(axis=1))
    dlo_t = np.maximum(dlo_t, 1)
    dhi_t = np.maximum(dhi_t, 1)
    tiles = [(int(dlo_t[t]), int(dhi_t[t])) for t in range(TPC)]
    slots = sum(128 * (dl + dh) for dl, dh in tiles)

    torder = sorted(range(TPC), key=lambda t: -(tiles[t][0] + tiles[t][1]))
    plan = {
        "tiles": tiles,
        "order": torder,
        "perms": perms,
        "pos": pos,
        "inflation": slots * R / E,
    }

    # per-core slot tables
    gidx_cores = []
    for c in range(R):
        own_lo = c * NPAD
        own = (pos[dst] >= own_lo) & (pos[dst] < own_lo + NPAD)
        s_own = src[own]
        d_own = dst[own]
        lpos = pos[d_own] - own_lo  # local permuted pos of dst in [0, NPC)
        hh = half_of[own]
        gkey = lpos * 2 + hh
        order = np.argsort(gkey, kind="stable")
        key_s = gkey[order]
        sp = pos[s_own][order]  # permuted global row of src
        first = np.searchsorted(key_s, key_s)
        rank = np.arange(len(key_s)) - first

        cols = []
        for t in torder:
            Dl, Dh = tiles[t]
            n0 = t * 128
            n1 = n0 + 128
            for h, D, base in ((0, Dl, 0), (1, Dh, HALF)):
                tab = np.full((128, D), SENT, dtype=np.int64)
                sel = (key_s % 2 == h) & (key_s // 2 >= n0) & (key_s // 2 < n1)
                rr = rank[sel]
                assert (rr < D).all(), "rank exceeded tile max degree"
                tab[key_s[sel] // 2 - n0, rr] = sp[sel] - base
                # gather layout: idx position (k*128 + p) -> partition p, block k
                flat = np.ascontiguousarray(tab.T.reshape(-1)).astype(np.int16)
                wrapped = flat.reshape(-1, 16)
                w = np.empty((128, 128 * D // 16), dtype=np.int16)
                for q in range(8):
                    w[q * 16 : (q + 1) * 16, :] = wrapped.T
                cols.append(w)
        gidx_cores.append(np.concatenate(cols, axis=1))
    plan["gidx"] = gidx_cores
    plan["W"] = gidx_cores[0].shape[1]
    assert all(g.shape[1] == plan["W"] for g in gidx_cores)
    return plan


# ---------------------------------------------------------------- bass kernel
def _build_bass(plan):
    import concourse.bacc as bacc
    import concourse.mybir as mybir
    import concourse.tile as tile
    from concourse.masks import make_identity

    f32 = mybir.dt.float32
    bf = mybir.dt.bfloat16
    i16 = mybir.dt.int16
    OP = mybir.AluOpType
    AF = mybir.ActivationFunctionType
    AX = mybir.AxisListType

    tiles = plan["tiles"]
    W = plan["W"]
    Ktmax = max(dl + dh for dl, dh in tiles)
    Dmax = max(max(dl, dh) for dl, dh in tiles)

    nc = bacc.Bacc(
        "TRN2",
        target_bir_lowering=False,
        debug=False,
        num_devices=R,
        num_swdge_queues=4,
        dynamic_dma_scratch_size=16384,
    )
    xT_in = nc.dram_tensor("xT", [128, NPAD], bf, kind="ExternalInput")
    gidx_in = nc.dram_tensor("gidx", [128, W], i16, kind="ExternalInput")
    wcat1_in = nc.dram_tensor("wcat1", [128, 136], bf, kind="ExternalInput")
    wcat2_in = nc.dram_tensor("wcat2", [128, 66], bf, kind="ExternalInput")
    b1_in = nc.dram_tensor("b1c", [128, 1], f32, kind="ExternalInput")
    b1n_in = nc.dram_tensor("b1n", [128, 1], f32, kind="ExternalInput")
    b2_in = nc.dram_tensor("b2r", [1, 64], f32, kind="ExternalInput")
    padneg_in = nc.dram_tensor("padneg", [128, 4], bf, kind="ExternalInput")
    csb_in = nc.dram_tensor("csb", [128, 2], f32, kind="ExternalInput")
    out_d = nc.dram_tensor("out", [NPAD, 64], f32, kind="ExternalOutput")

    with tile.TileContext(nc) as tc:
        with (
            tc.tile_pool(name="const", bufs=1) as cp,
            tc.tile_pool(name="work", bufs=5) as wp,
            tc.tile_pool(name="bigw", bufs=3) as wb,
            tc.tile_pool(name="bigw2", bufs=2) as wb2,
            tc.tile_pool(name="gath", bufs=5) as gp,
            tc.tile_pool(name="psum", bufs=2, space="PSUM") as pp,
            tc.tile_pool(name="psumA", bufs=4, space="PSUM") as ppa,
            tc.tile_pool(name="dram", bufs=1, space="DRAM") as dp,
        ):
            shard1 = dp.tile([NPAD, EL1], bf)
            table1 = dp.tile([R * NPAD, EL1], bf, addr_space="Shared")
            shard2 = dp.tile([NPAD, EL2], bf)
            table2 = dp.tile([R * NPAD, EL2], bf, addr_space="Shared")

            wcat1 = cp.tile([128, 136], bf)
            nc.sync.dma_start(out=wcat1[:], in_=wcat1_in[:])
            wcat2 = cp.tile([128, 66], bf)
            nc.sync.dma_start(out=wcat2[:], in_=wcat2_in[:])
            b1c = cp.tile([128, 1], f32)
            nc.sync.dma_start(out=b1c[:], in_=b1_in[:])
            b1n = cp.tile([128, 1], f32)
            nc.sync.dma_start(out=b1n[:], in_=b1n_in[:])
            b2p = cp.tile([1, 64], f32)
            nc.sync.dma_start(out=b2p[:1, :], in_=b2_in[:])
            b2b = cp.tile([128, 64], f32)
            nc.gpsimd.partition_broadcast(b2b[:], b2p[:1, :])
            ident = cp.tile([128, 128], f32)
            make_identity(nc, ident[:])
            idxall = cp.tile([128, W], i16)
            nc.sync.dma_start(out=idxall[:], in_=gidx_in[:])
            sh1 = cp.tile([128, TPC * RW1], bf)
            sh2 = cp.tile([128, TPC * RW2], bf)
            adb1 = cp.tile([128, 4 * TPC], bf)
            adb2 = cp.tile([128, TPC], bf)
            padneg = cp.tile([128, 4], bf)
            nc.sync.dma_start(out=padneg[:], in_=padneg_in[:])
            csb = cp.tile([128, 2], f32)
            nc.sync.dma_start(out=csb[:], in_=csb_in[:])

            # ones channels: sh1 col t*136 + h*33 + 32; sh2 col t*66 + 64
            nc.vector.memset(
                sh1[:]
                .rearrange("p (t r) -> p t r", r=RW1)[:, :, 0:132]
                .rearrange("p t (h c) -> p t h c", c=33)[:, :, :, 32:33],
                1.0,
            )
            nc.vector.memset(
                sh2[:].rearrange("p (t r) -> p t r", r=RW2)[:, :, 64:65], 1.0
            )

            # ---------------- phase A: per owned tile h1 = x @ [W1|Ws1|Wd1]
            for t in range(TPC):
                xt = wp.tile([128, 128], bf, tag="xt")
                nc.sync.dma_start(out=xt[:], in_=xT_in[:, t * 128 : (t + 1) * 128])
                psA = ppa.tile([128, 136], f32, tag="psA")
                nc.tensor.matmul(
                    psA[:], lhsT=xt[:], rhs=wcat1[:], start=True, stop=True
                )
                # h -> interleaved 4x(32ch | skip ones), as -> cols 132:136
                nc.scalar.copy(
                    out=sh1[:]
                    .rearrange("p (t r) -> p t r", r=RW1)[:, t : t + 1, 0:132]
                    .rearrange("p t (h c) -> p t h c", c=33)[:, :, :, 0:32],
                    in_=psA[:, 0:128]
                    .rearrange("p (h c) -> p h c", h=4)
                    .unsqueeze(1),
                )
                nc.scalar.copy(
                    out=sh1[:, t * RW1 + 132 : t * RW1 + 136], in_=psA[:, 128:132]
                )
                nc.scalar.copy(out=adb1[:, 4 * t : 4 * t + 4], in_=psA[:, 132:136])
                if t == 48:
                    # pad rows: alpha_src = -1e30 so pad slots vanish
                    nc.vector.tensor_tensor(
                        out=sh1[:, 48 * RW1 + 132 : 48 * RW1 + 136],
                        in0=sh1[:, 48 * RW1 + 132 : 48 * RW1 + 136],
                        in1=padneg[:],
                        op=OP.add,
                    )
                nc.sync.dma_start(
                    out=shard1[:].rearrange("(t p) r -> p t r", p=128)[
                        :, t : t + 1, 0:RW1
                    ],
                    in_=sh1[:]
                    .rearrange("p (t r) -> p t r", r=RW1)[:, t : t + 1, :],
                )

            nc.gpsimd.collective_compute(
                "AllGather",
                mybir.AluOpType.bypass,
                replica_groups=[list(range(R))],
                ins=[shard1.opt()],
                outs=[table1.opt()],
            )

            # batched self-terms, layer 1: w_self[p,(t,h)] = exp(prelu(as+ad))
            eself1 = cp.tile([128, 4 * TPC], bf)
            nc.vector.tensor_tensor(
                out=eself1[:].rearrange("p (t h) -> p t h", h=4),
                in0=sh1[:].rearrange("p (t r) -> p t r", r=RW1)[:, :, 132:136],
                in1=adb1[:].rearrange("p (t h) -> p t h", h=4),
                op=OP.add,
            )
            wself1 = cp.tile([128, 4 * TPC], bf)
            nc.scalar.activation(wself1[:], eself1[:], AF.Prelu, alpha=NEG_SLOPE)
            nc.scalar.activation(wself1[:], wself1[:], AF.Exp)
            nc.vector.tensor_scalar_max(wself1[:], wself1[:], 1e-16)

            col = [0]  # running idx column offset
            qrr = [0]  # SWDGE queue round-robin
            ni_regs = {
                128 * b: nc.gpsimd.to_reg(128 * b) for b in range(1, 9)
            }

            def gather_tile(table, EL, Dl, Dh):
                # one buffer per tile: lo blocks [0,Dl), hi blocks [Dl,Kt)
                g = gp.tile([128, Ktmax * EL1], bf, tag="g")
                for D, base0, base1, coff in (
                    (Dl, 0, HALF, 0),
                    (Dh, HALF, R * NPAD, Dl),
                ):
                    splits = -(-D // 8)  # sub-gathers <= 1024 rows
                    c0 = 0
                    for s in range(splits):
                        c1 = D * (s + 1) // splits
                        NI = 128 * (c1 - c0)
                        nc.gpsimd.dma_gather(
                            g[:, (coff + c0) * EL : (coff + c1) * EL].rearrange(
                                "p (c r) -> p c r", r=EL
                            ),
                            table[base0:base1, :],
                            idxall[:, col[0] : col[0] + NI // 16],
                            NI,
                            ni_regs[NI],
                            EL,
                            single_packet=True,
                            queue_num=qrr[0] % 4,
                        )
                        qrr[0] += 1
                        col[0] += NI // 16
                        c0 = c1
                return g

            # ---------------- phase B: layer-1 attention + aggregation
            # descending-Kt order: big tiles first, so the phase tail (and
            # the AllGather2 gate) drains through small, fast tiles
            for t in plan["order"]:
                Dl, Dh = tiles[t]
                Kt = Dl + Dh
                g = gather_tile(table1, EL1, Dl, Dh)
                # e[p,(h,k)] = as[src] + ad[dst]  (h-major so the multiply's
                # weight operand is k-contiguous)
                ebuf = wp.tile([128, 4 * Kt], bf, tag="ebuf")
                nc.vector.tensor_tensor(
                    out=ebuf[:].rearrange("p (h k) -> p h k", k=Kt),
                    in0=g[:]
                    .rearrange("p (k r) -> p k r", r=EL1)[:, 0:Kt, 132:136]
                    .rearrange("p k h -> p h k"),
                    in1=adb1[:, 4 * t : 4 * t + 4]
                    .unsqueeze(2)
                    .to_broadcast([128, 4, Kt]),
                    op=OP.add,
                )
                lbuf = wp.tile([128, 4 * Kt], bf, tag="lbuf")
                nc.scalar.activation(lbuf[:], ebuf[:], AF.Prelu, alpha=NEG_SLOPE)
                exb = wp.tile([128, 4 * Kt], bf, tag="exb")
                nc.scalar.activation(exb[:], lbuf[:], AF.Exp)

                # scr[p, (h,33,k)] = payload * w ; slot k=Kt = self-term
                # (k innermost so the reduce is contiguous; the multiply pays
                # the stride on in0, which measures cheaper than a strided
                # reduce)
                K1 = Kt + 1
                scr = wb.tile([128, (Ktmax + 1) * 132], bf, tag="scr")
                exv = exb[:].rearrange("p (h k) -> p h k", k=Kt)
                nc.vector.tensor_tensor(
                    out=scr[:, 0 : K1 * 132].rearrange(
                        "p (h c k) -> p h c k", h=4, c=33
                    )[:, :, :, 0:Kt],
                    in0=g[:]
                    .rearrange("p (k r) -> p k r", r=EL1)[:, 0:Kt, 0:132]
                    .rearrange("p k (h c) -> p h c k", h=4),
                    in1=exv[:]
                    .unsqueeze(2)
                    .to_broadcast([128, 4, 33, Kt]),
                    op=OP.mult,
                )
                nc.vector.tensor_tensor(
                    out=scr[:, 0 : K1 * 132].rearrange(
                        "p (h c k) -> p h c k", h=4, c=33
                    )[:, :, :, Kt : Kt + 1],
                    in0=sh1[:, t * RW1 : t * RW1 + 132]
                    .rearrange("p (h c) -> p h c", h=4)
                    .unsqueeze(3),
                    in1=wself1[:, 4 * t : 4 * t + 4]
                    .unsqueeze(2)
                    .unsqueeze(3)
                    .to_broadcast([128, 4, 33, 1]),
                    op=OP.mult,
                )
                # raw[p, 132] = sum_k scr (incl. self slot); den at h*33+32
                raw = wp.tile([128, 132], f32, tag="raw")
                nc.vector.reduce_sum(
                    out=raw[:],
                    in_=scr[:, 0 : K1 * 132].rearrange("p (r k) -> p r k", k=K1),
                    axis=AX.X,
                )
                rden = wp.tile([128, 4], f32, tag="rden")
                nc.vector.reciprocal(
                    rden[:].unsqueeze(2),
                    raw[:].rearrange("p (h c) -> p h c", h=4)[:, :, 32:33],
                )
                out1 = wp.tile([128, 128], f32, tag="out1")
                for h in range(4):
                    nc.scalar.activation(
                        out1[:, 32 * h : 32 * h + 32],
                        raw[:, 33 * h : 33 * h + 32],
                        AF.Identity,
                        scale=rden[:, h : h + 1],
                    )
                # transpose -> [c, n], ELU(z + b1) = relu(z)+exp(-relu(-z)),
                # then @ [W2|Ws2|Wd2] (the "-1" folded into b2eff/csb)
                psT = pp.tile([128, 128], f32, tag="psT")
                nc.tensor.transpose(psT[:], out1[:], ident[:])
                rt = wp.tile([128, 128], bf, tag="rt")
                nc.scalar.activation(rt[:], psT[:], AF.Relu, bias=b1c[:, :1])
                mt = wp.tile([128, 128], f32, tag="mt")
                nc.scalar.activation(
                    mt[:], psT[:], AF.Relu, scale=-1.0, bias=b1n[:, :1]
                )
                emt = wp.tile([128, 128], bf, tag="emt")
                nc.scalar.activation(emt[:], mt[:], AF.Exp, scale=-1.0)
                ps2 = pp.tile([128, 66], f32, tag="ps2")
                nc.tensor.matmul(
                    ps2[:], lhsT=rt[:], rhs=wcat2[:], start=True, stop=False
                )
                nc.tensor.matmul(
                    ps2[:], lhsT=emt[:], rhs=wcat2[:], start=False, stop=True
                )
                nc.scalar.copy(out=sh2[:, t * RW2 : t * RW2 + 64], in_=ps2[:, 0:64])
                nc.scalar.activation(
                    sh2[:, t * RW2 + 65 : t * RW2 + 66],
                    ps2[:, 64:65],
                    AF.Identity,
                    bias=csb[:, 0:1],
                )
                nc.scalar.activation(
                    adb2[:, t : t + 1],
                    ps2[:, 65:66],
                    AF.Identity,
                    bias=csb[:, 1:2],
                )
                if t == 48:
                    nc.vector.tensor_tensor(
                        out=sh2[:, 48 * RW2 + 65 : 48 * RW2 + 66],
                        in0=sh2[:, 48 * RW2 + 65 : 48 * RW2 + 66],
                        in1=padneg[:, 0:1],
                        op=OP.add,
                    )
                nc.sync.dma_start(
                    out=shard2[:].rearrange("(t p) r -> p t r", p=128)[
                        :, t : t + 1, 0:RW2
                    ],
                    in_=sh2[:]
                    .rearrange("p (t r) -> p t r", r=RW2)[:, t : t + 1, :],
                )
            nc.gpsimd.collective_compute(
                "AllGather",
                mybir.AluOpType.bypass,
                replica_groups=[list(range(R))],
                ins=[shard2.opt()],
                outs=[table2.opt()],
            )

            # batched self-terms, layer 2
            eself2 = cp.tile([128, TPC], bf)
            nc.vector.tensor_tensor(
                out=eself2[:].unsqueeze(2),
                in0=sh2[:].rearrange("p (t r) -> p t r", r=RW2)[:, :, 65:66],
                in1=adb2[:].unsqueeze(2),
                op=OP.add,
            )
            wself2 = cp.tile([128, TPC], bf)
            nc.scalar.activation(wself2[:], eself2[:], AF.Prelu, alpha=NEG_SLOPE)
            nc.scalar.activation(wself2[:], wself2[:], AF.Exp)
            nc.vector.tensor_scalar_max(wself2[:], wself2[:], 1e-16)

            # ---------------- phase C: layer-2 attention + aggregation
            col2 = col[0]
            col[0] = 0
            for t in plan["order"]:
                Dl, Dh = tiles[t]
                Kt = Dl + Dh
                g = gather_tile(table2, EL2, Dl, Dh)
                e2 = wp.tile([128, Kt], bf, tag="e2")
                nc.vector.tensor_tensor(
                    out=e2[:].unsqueeze(2),
                    in0=g[:].rearrange("p (k r) -> p k r", r=EL2)[:, 0:Kt, 65:66],
                    in1=adb2[:, t : t + 1]
                    .unsqueeze(1)
                    .to_broadcast([128, Kt, 1]),
                    op=OP.add,
                )
                l2b = wp.tile([128, Kt], bf, tag="l2b")
                nc.scalar.activation(l2b[:], e2[:], AF.Prelu, alpha=NEG_SLOPE)
                ex2 = wp.tile([128, Kt], bf, tag="ex2")
                nc.scalar.activation(ex2[:], l2b[:], AF.Exp)

                K1 = Kt + 1
                scr2 = wb2.tile([128, (Ktmax + 1) * 65], bf, tag="scr2")
                nc.vector.tensor_tensor(
                    out=scr2[:, 0 : K1 * 65].rearrange("p (c k) -> p c k", c=65)[
                        :, :, 0:Kt
                    ],
                    in0=g[:]
                    .rearrange("p (k r) -> p k r", r=EL2)[:, 0:Kt, 0:65]
                    .rearrange("p k c -> p c k"),
                    in1=ex2[:]
                    .unsqueeze(1)
                    .to_broadcast([128, 65, Kt]),
                    op=OP.mult,
                )
                nc.vector.tensor_tensor(
                    out=scr2[:, 0 : K1 * 65].rearrange("p (c k) -> p c k", c=65)[
                        :, :, Kt : Kt + 1
                    ],
                    in0=sh2[:, t * RW2 : t * RW2 + 65].unsqueeze(2),
                    in1=wself2[:, t : t + 1]
                    .unsqueeze(2)
                    .to_broadcast([128, 65, 1]),
                    op=OP.mult,
                )
                raw2 = wp.tile([128, 65], f32, tag="raw2")
                nc.vector.reduce_sum(
                    out=raw2[:],
                    in_=scr2[:, 0 : K1 * 65].rearrange("p (c k) -> p c k", k=K1),
                    axis=AX.X,
                )
                rden2 = wp.tile([128, 1], f32, tag="rden2")
                nc.vector.reciprocal(rden2[:], raw2[:, 64:65])
                outst = wp.tile([128, 64], f32, tag="outst")
                nc.vector.scalar_tensor_tensor(
                    out=outst[:],
                    in0=raw2[:, 0:64],
                    scalar=rden2[:, 0:1],
                    in1=b2b[:],
                    op0=OP.mult,
                    op1=OP.add,
                )
                nc.sync.dma_start(
                    out=out_d[t * 128 : (t + 1) * 128, :], in_=outst[:]
                )
            assert col[0] == col2

    nc.finalize()
    return nc


# ---------------------------------------------------------------- entry point
_cache = {}


def kernel(x, edge_index, W1, att_src1, att_dst1, b1, W2, att_src2, att_dst2, b2):
    import ml_dtypes

    from concourse.bass_utils import run_bass_kernel_spmd

    BF = ml_dtypes.bfloat16
    x = np.asarray(x, dtype=np.float32)
    edge_index = np.asarray(edge_index, dtype=np.int64)
    W1 = np.asarray(W1, dtype=np.float32)
    W2 = np.asarray(W2, dtype=np.float32)
    att_src1 = np.asarray(att_src1, dtype=np.float32)
    att_dst1 = np.asarray(att_dst1, dtype=np.float32)
    att_src2 = np.asarray(att_src2, dtype=np.float32)
    att_dst2 = np.asarray(att_dst2, dtype=np.float32)
    b1 = np.asarray(b1, dtype=np.float32)
    b2 = np.asarray(b2, dtype=np.float32)

    key = hash(edge_index.tobytes())
    if "plan" not in _cache or _cache.get("key") != key:
        _cache["plan"] = _build_plan(edge_index)
        _cache["nc"] = _build_bass(_cache["plan"])
        _cache["key"] = key
    plan = _cache["plan"]
    nc = _cache["nc"]

    # weight packing: as = x @ (W1 . att_src) etc.
    W1r = W1.reshape(IN_CH, HEADS, HIDDEN)
    Ws1 = np.einsum("khc,hc->kh", W1r, att_src1)  # [128, 4]
    Wd1 = np.einsum("khc,hc->kh", W1r, att_dst1)
    wcat1 = np.concatenate([W1, Ws1, Wd1], axis=1).astype(BF)  # [128, 136]
    Ws2 = W2 @ att_src2[0]  # [128]
    Wd2 = W2 @ att_dst2[0]
    wcat2 = np.concatenate([W2, Ws2[:, None], Wd2[:, None]], axis=1).astype(BF)
    cs = wcat2.astype(np.float32).sum(axis=0)  # ELU+1 correction: colsums
    b2eff = (b2 - cs[0:64]).astype(np.float32)

    csb_host = np.zeros((128, 2), dtype=np.float32)
    csb_host[:, 0] = -cs[64]
    csb_host[:, 1] = -cs[65]
    padneg_host = np.zeros((128, 4), dtype=BF)
    padneg_host[NPC % 128 :] = BF(-1e30)
    in_maps = []
    for c in range(R):
        xp = np.zeros((NPAD, IN_CH), dtype=np.float32)
        xp[:NPC] = x[plan["perms"][c]]
        in_maps.append(
            {
                "xT": np.ascontiguousarray(xp.T).astype(BF),
                "gidx": plan["gidx"][c],
                "wcat1": wcat1,
                "wcat2": wcat2,
                "b1c": b1.reshape(128, 1).astype(np.float32),
                "b1n": (-b1).reshape(128, 1).astype(np.float32),
                "b2r": b2eff.reshape(1, 64),
                "padneg": padneg_host,
                "csb": csb_host,
            }
        )

    res = run_bass_kernel_spmd(nc, in_maps, core_ids=list(range(R)))
    _cache["last_res"] = res
    out = np.empty((N, OUT_CH), dtype=np.float32)
    for c in range(R):
        out[plan["perms"][c]] = res.results[c]["out"][:NPC]
    return out
